# revision 25
# baseline (speedup 1.0000x reference)
import os
import numpy as np
import ml_dtypes

# nn_MultiHeadAttention: B=4, S=2048, D=1024, HEADS=16, DIM_HEAD=64.
# Sharding: batch (4) x head-group (2) across 8 cores. Each core computes
# attention for one batch and 8 heads, plus its partial of the output
# projection (row-parallel W0); the two head-group partials per batch are
# summed on the host.
#
# v2: overlap-restructured. The kernel is ACT(exp)-bound in steady state
# (256 activations x ~1.1us = 285us of ScalarE); the goal is to hide all
# PE work (projections, V-prep, output projection) inside that window.
#  - DMA emission in dependency order so the first scores/exp can start
#    ~15us in (wk, kv-tb0, wv, wq, qT-ibl0 first).
#  - V-prep (vp), K/Q projections and out-projection run as fine-grained
#    "extras" (half-blocks, ~0.85us) inside attention jg slots, on
#    explicitly managed spare PSUM banks (no tag-rotation collisions
#    with the attention pipeline).
#  - PE warmup matmuls during the initial DMA wait (HAM clock gate).
#  - Normalization copies PSUM->SBUF first to release banks early.
B, S, D = 4, 2048, 1024
HEADS, DH = 16, 64
HPC = 8               # heads per core
E = HPC * DH          # 512 local projection channels
SCALE = DH ** -0.5
P = 128
MT = D // P           # 8 contraction tiles
NPAIR = HPC // 2      # 4 head pairs (= e-chunks of 128)
NTB = S // 512        # 4 query blocks of 512
NJT = S // P          # 16 key tiles of 128
VPW = 65 + 128        # vp columns per pair: h0 [V|1], h1 [1|0*63|V]

_CACHE = {}


def _build():
    if "nc" in _CACHE:
        return _CACHE["nc"]
    import concourse.bacc as bacc
    import concourse.mybir as mybir
    from concourse.tile import TileContext

    f32 = mybir.dt.float32
    bf16 = mybir.dt.bfloat16
    EXP = mybir.ActivationFunctionType.Exp
    MULT = mybir.AluOpType.mult

    nc = bacc.Bacc("TRN2", target_bir_lowering=False, debug=False, num_devices=8)
    qT_d = nc.dram_tensor("qT", [D, S], bf16, kind="ExternalInput")
    kvT_d = nc.dram_tensor("kvT", [D, S], bf16, kind="ExternalInput")
    wq_d = nc.dram_tensor("wqT", [D, E], bf16, kind="ExternalInput")
    wk_d = nc.dram_tensor("wkT", [D, E], bf16, kind="ExternalInput")
    wv_d = nc.dram_tensor("wvT", [D, E], bf16, kind="ExternalInput")
    w0_d = nc.dram_tensor("w0a", [E, D], bf16, kind="ExternalInput")
    out_d = nc.dram_tensor("poutT", [D, S], f32, kind="ExternalOutput")

    with TileContext(nc) as tc:
        with (
            tc.tile_pool(name="pers", bufs=1) as pers,
            tc.tile_pool(name="psS", bufs=1, space="PSUM") as psS,
            tc.tile_pool(name="psX", bufs=1, space="PSUM") as psX,
        ):
            # ---- persistent SBUF tiles ----
            w0a = [pers.tile([P, D], bf16, tag=f"w0{p}", name=f"w0{p}") for p in range(NPAIR)]
            qpt = [pers.tile([P, S], bf16, tag=f"qp{p}", name=f"qp{p}") for p in range(NPAIR)]
            kpt = [pers.tile([P, S], bf16, tag=f"kp{p}", name=f"kp{p}") for p in range(NPAIR)]
            vp = [pers.tile([P, NPAIR * VPW], bf16, tag=f"vp{t}", name=f"vp{t}") for t in range(NJT)]
            onorm = [pers.tile([P, S], bf16, tag=f"on{p}", name=f"on{p}") for p in range(NPAIR)]
            warm = pers.tile([P, P], bf16, tag="warm", name="warm")
            warm2 = pers.tile([P, 512], bf16, tag="warm2", name="warm2")

            # 4 explicitly-managed PSUM banks (+ 4 for the scores pipeline).
            # Window widx uses pA{widx%2}/pB{widx%2} for its AV accumulators;
            # interleaved extras use the spare parity's banks.
            def xps(tag):
                return psX.tile([P, 512], f32, tag=tag, name=tag, bufs=1)

            with tc.tile_pool(name="phA", bufs=1) as pha:
                # One big tile per input tensor; the per-mt "tiles" are views.
                # Each logical slice lands as ONE dma_start with a 3D access
                # pattern -- 11 input DMA issues instead of ~90, so the sync
                # queue's ~0.7us-per-issue serialization stops gating the head.
                qbig = pha.tile([P, MT * S], bf16, tag="qTb", name="qTb")
                kvbig = pha.tile([P, MT * S], bf16, tag="kvTb", name="kvTb")
                wqbig = pha.tile([P, MT * E], bf16, tag="wqb", name="wqb")
                wkbig = pha.tile([P, MT * E], bf16, tag="wkb", name="wkb")
                wvbig = pha.tile([P, MT * E], bf16, tag="wvb", name="wvb")
                qTt = [qbig[:, i * S:(i + 1) * S] for i in range(MT)]
                kvTt = [kvbig[:, i * S:(i + 1) * S] for i in range(MT)]
                wqt = [wqbig[:, i * E:(i + 1) * E] for i in range(MT)]
                wkt = [wkbig[:, i * E:(i + 1) * E] for i in range(MT)]
                wvt = [wvbig[:, i * E:(i + 1) * E] for i in range(MT)]

                def slab(big, dram, width, c0, c1):
                    # one DMA for columns [c0:c1) of all MT row-blocks
                    dst = big.rearrange("x (i c) -> x i c", c=width)[:, :, c0:c1]
                    src = dram.rearrange("(i x) c -> x i c", x=P)[:, :, c0:c1]
                    nc.sync.dma_start(out=dst, in_=src)

                slab(wkbig, wk_d, E, 0, P)        # wk ec0
                slab(kvbig, kvT_d, S, 0, 512)     # kv tb0
                slab(wvbig, wv_d, E, 0, E)        # wv (vp path)
                slab(wqbig, wq_d, E, 0, P)        # wq ec0
                slab(qbig, qT_d, S, 0, 512)       # qT ibl0
                slab(kvbig, kvT_d, S, 512, 1024)  # kv tb1
                slab(kvbig, kvT_d, S, 1024, S)    # kv tb2/3
                slab(qbig, qT_d, S, 512, S)       # qT rest
                slab(wkbig, wk_d, E, P, E)        # wk rest
                slab(wqbig, wq_d, E, P, E)        # wq rest
                for p in range(NPAIR):
                    nc.sync.dma_start(out=w0a[p][:, :], in_=w0_d[p * P:(p + 1) * P, :])

                # ---- PE warmup during the DMA wait (HAM clock gate).
                # Two batches so the K projection can slot in between. ----
                nc.vector.memset(warm[:, :], 0.0)
                nc.vector.memset(warm2[:, :], 0.0)
                wps = xps("pB1")
                for r in range(32):
                    nc.tensor.matmul(wps[:, :], lhsT=warm[:, :], rhs=warm2[:, :],
                                     start=True, stop=True)

                def warmup2():
                    wps2 = xps("pB1")
                    for r in range(12):
                        nc.tensor.matmul(wps2[:, :], lhsT=warm[:, :], rhs=warm2[:, :],
                                         start=True, stop=True)

                with (
                    tc.tile_pool(name="at", bufs=4) as atp,
                    tc.tile_pool(name="small", bufs=2) as small,
                    tc.tile_pool(name="ob", bufs=3) as obp,
                ):
                    # ---------- building blocks (emitted as half-units) ----------
                    def make_proj_halves(dst, wt, xt, ec, tb, tag):
                        state = {}

                        def h0():
                            ps = xps(tag)
                            state["ps"] = ps
                            for mt in range(MT // 2):
                                nc.tensor.matmul(
                                    ps[:, :],
                                    lhsT=wt[mt][:, ec * P:(ec + 1) * P],
                                    rhs=xt[mt][:, tb * 512:(tb + 1) * 512],
                                    start=(mt == 0), stop=False)

                        def h1():
                            ps = state["ps"]
                            for mt in range(MT // 2, MT):
                                nc.tensor.matmul(
                                    ps[:, :],
                                    lhsT=wt[mt][:, ec * P:(ec + 1) * P],
                                    rhs=xt[mt][:, tb * 512:(tb + 1) * 512],
                                    start=False, stop=(mt == MT - 1))
                            nc.vector.tensor_copy(
                                out=dst[ec][:, tb * 512:(tb + 1) * 512], in_=ps[:, :])
                        return [h0, h1]

                    def make_vp_halves(t, tag):
                        state = {}

                        def h0():
                            nc.gpsimd.memset(vp[t][:, :], 0.0)
                            v3 = vp[t].rearrange("x (g c) -> x g c", c=VPW)
                            state["v3"] = v3
                            nc.gpsimd.memset(v3[:, :, 64:66], 1.0)
                            ps = xps(tag)
                            state["ps"] = ps
                            for mt in range(MT // 2):
                                nc.tensor.matmul(
                                    ps[:, :],
                                    lhsT=kvTt[mt][:, t * P:(t + 1) * P],
                                    rhs=wvt[mt][:, :],
                                    start=(mt == 0), stop=False)

                        def h1():
                            ps = state["ps"]
                            v3 = state["v3"]
                            for mt in range(MT // 2, MT):
                                nc.tensor.matmul(
                                    ps[:, :],
                                    lhsT=kvTt[mt][:, t * P:(t + 1) * P],
                                    rhs=wvt[mt][:, :],
                                    start=False, stop=(mt == MT - 1))
                            p3 = ps.rearrange("x (g c) -> x g c", c=P)
                            nc.vector.tensor_copy(out=v3[:, :, 0:64], in_=p3[:, :, 0:64])
                            nc.vector.tensor_copy(out=v3[:, :, 129:193], in_=p3[:, :, 64:128])
                        return [h0, h1]

                    def make_final_unit(dc, tb, tag, eng=None):
                        def u():
                            pp = xps(tag)
                            for p in range(NPAIR):
                                nc.tensor.matmul(
                                    pp[:, :],
                                    lhsT=w0a[p][:, dc * P:(dc + 1) * P],
                                    rhs=onorm[p][:, tb * 512:(tb + 1) * 512],
                                    start=(p == 0), stop=(p == NPAIR - 1))
                            ob = obp.tile([P, 512], f32, tag="ob", name="ob", bufs=3)
                            nc.vector.tensor_copy(out=ob[:, :], in_=pp[:, :])
                            (eng or nc.sync).dma_start(
                                out=out_d[dc * P:(dc + 1) * P, tb * 512:(tb + 1) * 512],
                                in_=ob[:, :])
                        return [u]

                    # ---------- extras schedule ----------
                    # extras[(widx, jg)] = list of thunks run after that jg's
                    # scores/exp/AV emission (so they never delay the exp of
                    # the jg they sit in; deadlines are checked one jg ahead).
                    extras = {}
                    spare_ctr = {}

                    def sched_block(widx, jg0, jg1, maker, *args):
                        par = 1 - (widx % 2)
                        sp = spare_ctr.setdefault(widx, [0])
                        tag = ("pA%d" % par) if sp[0] % 2 == 0 else ("pB%d" % par)
                        sp[0] += 1
                        hs = maker(*args, tag)
                        if len(hs) == 1:
                            extras.setdefault((widx, jg0), []).append(hs[0])
                        else:
                            extras.setdefault((widx, jg0), []).append(hs[0])
                            extras.setdefault((widx, jg1), []).append(hs[1])

                    # widx0: JIT vp[2..15], K-ec0 tb2/3, Q-ec0 ibl1 (K-ec0 tb1
                    # lives in the head). With lookahead-2 emission, extras at
                    # slot k run after AV(k) and after scores(k+2): vp for AV
                    # jg m must sit at slot <= m-1; kpt tb_t (scores jg 2t,
                    # emitted at stage 2t-2) at slot <= 2t-3.
                    sched_block(0, 0, 0, make_vp_halves, 2)
                    sched_block(0, 0, 0, make_vp_halves, 3)
                    sched_block(0, 1, 1, make_vp_halves, 4)
                    sched_block(0, 1, 1, make_vp_halves, 5)
                    sched_block(0, 1, 1, make_proj_halves, kpt, wkt, kvTt, 0, 2)
                    sched_block(0, 2, 2, make_vp_halves, 6)
                    sched_block(0, 2, 2, make_vp_halves, 7)
                    sched_block(0, 3, 3, make_vp_halves, 8)
                    sched_block(0, 3, 3, make_vp_halves, 9)
                    sched_block(0, 3, 3, make_proj_halves, kpt, wkt, kvTt, 0, 3)
                    sched_block(0, 4, 4, make_vp_halves, 10)
                    sched_block(0, 4, 4, make_vp_halves, 11)
                    sched_block(0, 5, 5, make_vp_halves, 12)
                    sched_block(0, 5, 5, make_vp_halves, 13)
                    sched_block(0, 5, 5, make_proj_halves, qpt, wqt, qTt, 0, 1)
                    sched_block(0, 6, 6, make_vp_halves, 14)
                    sched_block(0, 6, 6, make_vp_halves, 15)

                    # widx1..3: Q-ec0 ibl2/3 + pair-1 K/Q projections
                    sched_block(1, 0, 1, make_proj_halves, qpt, wqt, qTt, 0, 2)
                    sched_block(1, 2, 3, make_proj_halves, kpt, wkt, kvTt, 1, 0)
                    sched_block(1, 4, 5, make_proj_halves, kpt, wkt, kvTt, 1, 1)
                    sched_block(1, 6, 7, make_proj_halves, kpt, wkt, kvTt, 1, 2)
                    sched_block(2, 0, 1, make_proj_halves, qpt, wqt, qTt, 0, 3)
                    sched_block(2, 2, 3, make_proj_halves, kpt, wkt, kvTt, 1, 3)
                    sched_block(2, 4, 5, make_proj_halves, qpt, wqt, qTt, 1, 0)
                    sched_block(2, 6, 7, make_proj_halves, qpt, wqt, qTt, 1, 1)
                    sched_block(3, 0, 1, make_proj_halves, qpt, wqt, qTt, 1, 2)
                    sched_block(3, 2, 3, make_proj_halves, qpt, wqt, qTt, 1, 3)

                    # widx4..6 (p1): pair-2 projections
                    sched_block(4, 0, 1, make_proj_halves, kpt, wkt, kvTt, 2, 0)
                    sched_block(4, 2, 3, make_proj_halves, kpt, wkt, kvTt, 2, 1)
                    sched_block(4, 4, 5, make_proj_halves, kpt, wkt, kvTt, 2, 2)
                    sched_block(4, 6, 7, make_proj_halves, kpt, wkt, kvTt, 2, 3)
                    sched_block(5, 0, 1, make_proj_halves, qpt, wqt, qTt, 2, 0)
                    sched_block(5, 2, 3, make_proj_halves, qpt, wqt, qTt, 2, 1)
                    sched_block(5, 4, 5, make_proj_halves, qpt, wqt, qTt, 2, 2)
                    sched_block(5, 6, 7, make_proj_halves, qpt, wqt, qTt, 2, 3)

                    # widx8..9 (p2): pair-3 projections
                    sched_block(8, 0, 1, make_proj_halves, kpt, wkt, kvTt, 3, 0)
                    sched_block(8, 2, 3, make_proj_halves, kpt, wkt, kvTt, 3, 1)
                    sched_block(8, 4, 5, make_proj_halves, kpt, wkt, kvTt, 3, 2)
                    sched_block(8, 6, 7, make_proj_halves, kpt, wkt, kvTt, 3, 3)
                    sched_block(9, 0, 1, make_proj_halves, qpt, wqt, qTt, 3, 0)
                    sched_block(9, 2, 3, make_proj_halves, qpt, wqt, qTt, 3, 1)
                    sched_block(9, 4, 5, make_proj_halves, qpt, wqt, qTt, 3, 2)
                    sched_block(9, 6, 7, make_proj_halves, qpt, wqt, qTt, 3, 3)

                    # widx13..15 (p3 i1..3): out-projection for tb0..2
                    for k, widx in enumerate((13, 14, 15)):
                        for dc in range(D // P):
                            sched_block(widx, dc, dc, make_final_unit, dc, k)

                    # ---------- head: minimal pre-work ----------
                    for h in make_proj_halves(kpt, wkt, kvTt, 0, 0, "pA1"):
                        h()
                    warmup2()
                    for h in make_proj_halves(qpt, wqt, qTt, 0, 0, "pA1"):
                        h()
                    head_tail = []
                    head_tail.extend(make_proj_halves(kpt, wkt, kvTt, 0, 1, "pB1"))
                    head_tail.extend(make_vp_halves(0, "pA1"))
                    head_tail.extend(make_vp_halves(1, "pB1"))

                    # ---------- software-pipelined attention ----------
                    # Per stage (widx, jg): emit scores+exp of the NEXT stage
                    # first (so the PE's in-order stream keeps ScalarE fed
                    # while AV of this stage waits on its exp), then the AV
                    # matmuls, then the interleaved extras, then (at jg==7)
                    # the normalization.
                    def emit_scores_exp(p, ibl, jg):
                        q0 = qpt[p]
                        out = []
                        for j in (2 * jg, 2 * jg + 1):
                            sAB = psS.tile([P, 1024], f32, tag="sAB", name="sAB", bufs=2)
                            nc.tensor.matmul(
                                sAB[:, 0:512],
                                lhsT=kpt[p][0:64, j * P:(j + 1) * P],
                                rhs=q0[0:64, ibl * 512:(ibl + 1) * 512],
                                start=True, stop=True,
                                tile_position=(0, 0),
                            )
                            nc.tensor.matmul(
                                sAB[:, 512:1024],
                                lhsT=kpt[p][64:128, j * P:(j + 1) * P],
                                rhs=q0[64:128, ibl * 512:(ibl + 1) * 512],
                                start=True, stop=True,
                                tile_position=(64, 0),
                            )
                            at = atp.tile([P, 1024], bf16, tag="at", name="at", bufs=6)
                            nc.scalar.activation(at[:, :], sAB[:, :], EXP, scale=SCALE)
                            out.append((j, at))
                        return out

                    powin = {}

                    def emit_avs(p, ibl, pend):
                        widx = p * NTB + ibl
                        if widx not in powin:
                            par = widx % 2
                            powin[widx] = (xps("pA%d" % par), xps("pB%d" % par))
                        po0, po1 = powin[widx]
                        vslc0 = (p * VPW, p * VPW + 65)
                        vslc1 = (p * VPW + 65, (p + 1) * VPW)
                        for j, at in pend:
                            nc.tensor.matmul(
                                po0[0:65, :],
                                lhsT=vp[j][:, vslc0[0]:vslc0[1]],
                                rhs=at[:, 0:512],
                                start=(j == 0), stop=(j == NJT - 1),
                            )
                            nc.tensor.matmul(
                                po1[:, :],
                                lhsT=vp[j][:, vslc1[0]:vslc1[1]],
                                rhs=at[:, 512:1024],
                                start=(j == 0), stop=(j == NJT - 1),
                            )

                    def emit_norm(p, ibl):
                        widx = p * NTB + ibl
                        po0, po1 = powin.pop(widx)
                        srow0 = small.tile([1, 512], f32, tag="srow0", name="srow0", bufs=1)
                        srow1 = small.tile([1, 512], f32, tag="srow1", name="srow1", bufs=1)
                        nc.vector.tensor_copy(out=srow0[:, :], in_=po0[64:65, :])
                        nc.vector.tensor_copy(out=srow1[:, :], in_=po1[0:1, :])
                        posb = small.tile([P, 512], f32, tag="posb", name="posb", bufs=1)
                        nc.vector.tensor_copy(out=posb[0:64, :], in_=po0[0:64, :])
                        nc.vector.tensor_copy(out=posb[64:128, :], in_=po1[64:128, :])
                        rr0 = small.tile([1, 512], f32, tag="rr0", name="rr0", bufs=1)
                        rr1 = small.tile([1, 512], f32, tag="rr1", name="rr1", bufs=1)
                        nc.vector.reciprocal_approx_fast(out=rr0[:, :], in_=srow0[:, :])
                        nc.vector.reciprocal_approx_fast(out=rr1[:, :], in_=srow1[:, :])
                        rbs = small.tile([P, 512], f32, tag="rbs", name="rbs", bufs=1)
                        rbt = small.tile([64, 512], f32, tag="rbt", name="rbt", bufs=1)
                        nc.gpsimd.partition_broadcast(rbs[0:64, :], rr0[0:1, :], channels=64)
                        nc.gpsimd.partition_broadcast(rbt[0:64, :], rr1[0:1, :], channels=64)
                        nc.sync.dma_start(out=rbs[64:128, :], in_=rbt[0:64, :])
                        nc.vector.tensor_tensor(
                            out=onorm[p][0:64, ibl * 512:(ibl + 1) * 512],
                            in0=posb[0:64, :], in1=rbs[0:64, :], op=MULT)
                        nc.vector.tensor_tensor(
                            out=onorm[p][64:128, ibl * 512:(ibl + 1) * 512],
                            in0=posb[64:128, :], in1=rbs[64:128, :], op=MULT)

                    stages = [(p, ibl, jg)
                              for p in range(NPAIR)
                              for ibl in range(NTB)
                              for jg in range(NJT // 2)]
                    pend = {0: emit_scores_exp(*stages[0]),
                            1: emit_scores_exp(*stages[1])}
                    for h in head_tail:
                        h()
                    for i, (p, ibl, jg) in enumerate(stages):
                        if i + 2 < len(stages):
                            pend[i + 2] = emit_scores_exp(*stages[i + 2])
                        emit_avs(p, ibl, pend.pop(i))
                        widx = p * NTB + ibl
                        for fn in extras.get((widx, jg), ()):
                            fn()
                        if jg == NJT // 2 - 1:
                            emit_norm(p, ibl)

                    # ---------- tail: out-projection for tb3, 4-bank spread,
                    # out-DMAs alternating across two DMA engines ----------
                    ttags = ("pA0", "pB0", "pA1", "pB1")
                    for dc in range(D // P):
                        eng = nc.scalar if dc % 2 == 0 else nc.sync
                        make_final_unit(dc, 3, ttags[dc % 4], eng)[0]()

    nc.compile()
    _CACHE["nc"] = nc
    return nc


def _prep_weights(Wq, Wkv, W0):
    bf = ml_dtypes.bfloat16
    per_group = {}
    for g in range(2):
        hg = np.arange(HPC) + g * HPC            # global head ids
        d = np.arange(DH)
        # e_local = h_l*64 + d ; reference maps: e_q = d*16+h, e_k = d*32+h,
        # e_v = d*32+16+h, out channel = h*64+d
        idx_q = (d[None, :] * HEADS + hg[:, None]).reshape(-1)
        idx_k = (d[None, :] * 2 * HEADS + hg[:, None]).reshape(-1)
        idx_v = (d[None, :] * 2 * HEADS + HEADS + hg[:, None]).reshape(-1)
        idx_o = (hg[:, None] * DH + d[None, :]).reshape(-1)
        per_group[g] = {
            "wqT": np.ascontiguousarray(Wq[idx_q, :].T).astype(bf),
            "wkT": np.ascontiguousarray(Wkv[idx_k, :].T).astype(bf),
            "wvT": np.ascontiguousarray(Wkv[idx_v, :].T).astype(bf),
            "w0a": np.ascontiguousarray(W0[:, idx_o].T).astype(bf),
        }
    return per_group


def kernel(q, kv, Wq, Wkv, W0):
    from concourse.bass_utils import run_bass_kernel_spmd

    q = np.asarray(q, dtype=np.float32)
    kv = np.asarray(kv, dtype=np.float32)
    Wq = np.asarray(Wq, dtype=np.float32)
    Wkv = np.asarray(Wkv, dtype=np.float32)
    W0 = np.asarray(W0, dtype=np.float32)

    nc = _build()
    bf = ml_dtypes.bfloat16
    wg = _prep_weights(Wq, Wkv, W0)
    in_maps = []
    for c in range(8):
        b, g = divmod(c, 2)
        in_maps.append({
            "qT": np.ascontiguousarray(q[b].T).astype(bf),
            "kvT": np.ascontiguousarray(kv[b].T).astype(bf),
            "wqT": wg[g]["wqT"],
            "wkT": wg[g]["wkT"],
            "wvT": wg[g]["wvT"],
            "w0a": wg[g]["w0a"],
        })
    trace = bool(int(os.environ.get("KERNEL_TRACE", "0")))
    res = run_bass_kernel_spmd(nc, in_maps, list(range(8)), trace=trace)
    _CACHE["last_result"] = res
    out = np.empty((B, S, D), dtype=np.float32)
    for b in range(B):
        acc = res.results[2 * b]["poutT"] + res.results[2 * b + 1]["poutT"]
        out[b] = acc.T
    return out


# revision 27
# speedup vs baseline: 1.1464x; 1.1464x over previous
import os
import numpy as np
import ml_dtypes

# nn_MultiHeadAttention: B=4, S=2048, D=1024, HEADS=16, DIM_HEAD=64.
# Sharding: batch (4) x head-group (2) across 8 cores. Each core computes
# attention for one batch and 8 heads, plus its partial of the output
# projection (row-parallel W0); the two head-group partials per batch are
# summed on the host.
#
# v2: overlap-restructured. The kernel is ACT(exp)-bound in steady state
# (256 activations x ~1.1us = 285us of ScalarE); the goal is to hide all
# PE work (projections, V-prep, output projection) inside that window.
#  - DMA emission in dependency order so the first scores/exp can start
#    ~15us in (wk, kv-tb0, wv, wq, qT-ibl0 first).
#  - V-prep (vp), K/Q projections and out-projection run as fine-grained
#    "extras" (half-blocks, ~0.85us) inside attention jg slots, on
#    explicitly managed spare PSUM banks (no tag-rotation collisions
#    with the attention pipeline).
#  - PE warmup matmuls during the initial DMA wait (HAM clock gate).
#  - Normalization copies PSUM->SBUF first to release banks early.
B, S, D = 4, 2048, 1024
HEADS, DH = 16, 64
HPC = 8               # heads per core
E = HPC * DH          # 512 local projection channels
SCALE = DH ** -0.5
P = 128
MT = D // P           # 8 contraction tiles
NPAIR = HPC // 2      # 4 head pairs (= e-chunks of 128)
NTB = S // 512        # 4 query blocks of 512
NJT = S // P          # 16 key tiles of 128
VPW = 65 + 128        # vp columns per pair: h0 [V|1], h1 [1|0*63|V]

_CACHE = {}


def _build():
    if "nc" in _CACHE:
        return _CACHE["nc"]
    import concourse.bacc as bacc
    import concourse.mybir as mybir
    from concourse.tile import TileContext

    f32 = mybir.dt.float32
    bf16 = mybir.dt.bfloat16
    EXP = mybir.ActivationFunctionType.Exp
    MULT = mybir.AluOpType.mult

    nc = bacc.Bacc("TRN2", target_bir_lowering=False, debug=False, num_devices=8)
    qT_d = nc.dram_tensor("qT", [D, S], bf16, kind="ExternalInput")
    kvT_d = nc.dram_tensor("kvT", [D, S], bf16, kind="ExternalInput")
    wq_d = nc.dram_tensor("wqT", [D, E], bf16, kind="ExternalInput")
    wk_d = nc.dram_tensor("wkT", [D, E], bf16, kind="ExternalInput")
    wv_d = nc.dram_tensor("wvT", [D, E], bf16, kind="ExternalInput")
    w0_d = nc.dram_tensor("w0a", [E, D], bf16, kind="ExternalInput")
    out_d = nc.dram_tensor("poutT", [D, S], f32, kind="ExternalOutput")

    with TileContext(nc) as tc:
        with (
            tc.tile_pool(name="pers", bufs=1) as pers,
            tc.tile_pool(name="psS", bufs=1, space="PSUM") as psS,
            tc.tile_pool(name="psX", bufs=1, space="PSUM") as psX,
        ):
            # ---- persistent SBUF tiles ----
            w0a = [pers.tile([P, D], bf16, tag=f"w0{p}", name=f"w0{p}") for p in range(NPAIR)]
            qpt = [pers.tile([P, S], bf16, tag=f"qp{p}", name=f"qp{p}") for p in range(NPAIR)]
            kpt = [pers.tile([P, S], bf16, tag=f"kp{p}", name=f"kp{p}") for p in range(NPAIR)]
            vp = [pers.tile([P, NPAIR * VPW], bf16, tag=f"vp{t}", name=f"vp{t}") for t in range(NJT)]
            onorm = [pers.tile([P, S], bf16, tag=f"on{p}", name=f"on{p}") for p in range(NPAIR)]
            warm = pers.tile([P, P], bf16, tag="warm", name="warm")
            warm2 = pers.tile([P, 512], bf16, tag="warm2", name="warm2")

            # 4 explicitly-managed PSUM banks (+ 4 for the scores pipeline).
            # Window widx uses pA{widx%2}/pB{widx%2} for its AV accumulators;
            # interleaved extras use the spare parity's banks.
            def xps(tag):
                return psX.tile([P, 512], f32, tag=tag, name=tag, bufs=1)

            with tc.tile_pool(name="phA", bufs=1) as pha:
                # One big tile per input tensor; the per-mt "tiles" are views.
                # Each logical slice lands as ONE dma_start with a 3D access
                # pattern -- 11 input DMA issues instead of ~90, so the sync
                # queue's ~0.7us-per-issue serialization stops gating the head.
                qbig = pha.tile([P, MT * S], bf16, tag="qTb", name="qTb")
                kvbig = pha.tile([P, MT * S], bf16, tag="kvTb", name="kvTb")
                wqbig = pha.tile([P, MT * E], bf16, tag="wqb", name="wqb")
                wkbig = pha.tile([P, MT * E], bf16, tag="wkb", name="wkb")
                wvbig = pha.tile([P, MT * E], bf16, tag="wvb", name="wvb")
                qTt = [qbig[:, i * S:(i + 1) * S] for i in range(MT)]
                kvTt = [kvbig[:, i * S:(i + 1) * S] for i in range(MT)]
                wqt = [wqbig[:, i * E:(i + 1) * E] for i in range(MT)]
                wkt = [wkbig[:, i * E:(i + 1) * E] for i in range(MT)]
                wvt = [wvbig[:, i * E:(i + 1) * E] for i in range(MT)]

                def slab(big, dram, width, c0, c1):
                    # per-mt-tile DMAs (one slab = 8 issues; big 3D-AP single
                    # DMAs stall the sync engine for ms-scale descriptor gen)
                    for i in range(MT):
                        nc.sync.dma_start(
                            out=big[:, i * width + c0:i * width + c1],
                            in_=dram[i * P:(i + 1) * P, c0:c1])

                slab(wkbig, wk_d, E, 0, P)        # wk ec0
                slab(kvbig, kvT_d, S, 0, 512)     # kv tb0
                slab(wvbig, wv_d, E, 0, E)        # wv (vp path)
                slab(wqbig, wq_d, E, 0, P)        # wq ec0
                slab(qbig, qT_d, S, 0, 512)       # qT ibl0
                slab(kvbig, kvT_d, S, 512, 1024)  # kv tb1
                slab(kvbig, kvT_d, S, 1024, S)    # kv tb2/3
                slab(qbig, qT_d, S, 512, S)       # qT rest
                slab(wkbig, wk_d, E, P, E)        # wk rest
                slab(wqbig, wq_d, E, P, E)        # wq rest
                for p in range(NPAIR):
                    nc.sync.dma_start(out=w0a[p][:, :], in_=w0_d[p * P:(p + 1) * P, :])

                # ---- PE warmup during the DMA wait (HAM clock gate).
                # Two batches so the K projection can slot in between. ----
                nc.vector.memset(warm[:, :], 0.0)
                nc.vector.memset(warm2[:, :], 0.0)
                wps = xps("pB1")
                for r in range(32):
                    nc.tensor.matmul(wps[:, :], lhsT=warm[:, :], rhs=warm2[:, :],
                                     start=True, stop=True)

                def warmup2():
                    wps2 = xps("pB1")
                    for r in range(12):
                        nc.tensor.matmul(wps2[:, :], lhsT=warm[:, :], rhs=warm2[:, :],
                                         start=True, stop=True)

                with (
                    tc.tile_pool(name="at", bufs=4) as atp,
                    tc.tile_pool(name="small", bufs=2) as small,
                    tc.tile_pool(name="ob", bufs=3) as obp,
                ):
                    # ---------- building blocks (emitted as half-units) ----------
                    def make_proj_halves(dst, wt, xt, ec, tb, tag):
                        state = {}

                        def h0():
                            ps = xps(tag)
                            state["ps"] = ps
                            for mt in range(MT // 2):
                                nc.tensor.matmul(
                                    ps[:, :],
                                    lhsT=wt[mt][:, ec * P:(ec + 1) * P],
                                    rhs=xt[mt][:, tb * 512:(tb + 1) * 512],
                                    start=(mt == 0), stop=False)

                        def h1():
                            ps = state["ps"]
                            for mt in range(MT // 2, MT):
                                nc.tensor.matmul(
                                    ps[:, :],
                                    lhsT=wt[mt][:, ec * P:(ec + 1) * P],
                                    rhs=xt[mt][:, tb * 512:(tb + 1) * 512],
                                    start=False, stop=(mt == MT - 1))
                            nc.vector.tensor_copy(
                                out=dst[ec][:, tb * 512:(tb + 1) * 512], in_=ps[:, :])
                        return [h0, h1]

                    def make_vp_halves(t, tag):
                        state = {}

                        def h0():
                            nc.gpsimd.memset(vp[t][:, :], 0.0)
                            v3 = vp[t].rearrange("x (g c) -> x g c", c=VPW)
                            state["v3"] = v3
                            nc.gpsimd.memset(v3[:, :, 64:66], 1.0)
                            ps = xps(tag)
                            state["ps"] = ps
                            for mt in range(MT // 2):
                                nc.tensor.matmul(
                                    ps[:, :],
                                    lhsT=kvTt[mt][:, t * P:(t + 1) * P],
                                    rhs=wvt[mt][:, :],
                                    start=(mt == 0), stop=False)

                        def h1():
                            ps = state["ps"]
                            v3 = state["v3"]
                            for mt in range(MT // 2, MT):
                                nc.tensor.matmul(
                                    ps[:, :],
                                    lhsT=kvTt[mt][:, t * P:(t + 1) * P],
                                    rhs=wvt[mt][:, :],
                                    start=False, stop=(mt == MT - 1))
                            p3 = ps.rearrange("x (g c) -> x g c", c=P)
                            nc.vector.tensor_copy(out=v3[:, :, 0:64], in_=p3[:, :, 0:64])
                            nc.vector.tensor_copy(out=v3[:, :, 129:193], in_=p3[:, :, 64:128])
                        return [h0, h1]

                    def make_final_unit(dc, tb, tag, eng=None):
                        def u():
                            pp = xps(tag)
                            for p in range(NPAIR):
                                nc.tensor.matmul(
                                    pp[:, :],
                                    lhsT=w0a[p][:, dc * P:(dc + 1) * P],
                                    rhs=onorm[p][:, tb * 512:(tb + 1) * 512],
                                    start=(p == 0), stop=(p == NPAIR - 1))
                            ob = obp.tile([P, 512], f32, tag="ob", name="ob", bufs=3)
                            nc.vector.tensor_copy(out=ob[:, :], in_=pp[:, :])
                            (eng or nc.sync).dma_start(
                                out=out_d[dc * P:(dc + 1) * P, tb * 512:(tb + 1) * 512],
                                in_=ob[:, :])
                        return [u]

                    # ---------- extras schedule ----------
                    # extras[(widx, jg)] = list of thunks run after that jg's
                    # scores/exp/AV emission (so they never delay the exp of
                    # the jg they sit in; deadlines are checked one jg ahead).
                    extras = {}
                    spare_ctr = {}

                    def sched_block(widx, jg0, jg1, maker, *args):
                        par = 1 - (widx % 2)
                        sp = spare_ctr.setdefault(widx, [0])
                        tag = ("pA%d" % par) if sp[0] % 2 == 0 else ("pB%d" % par)
                        sp[0] += 1
                        hs = maker(*args, tag)
                        if len(hs) == 1:
                            extras.setdefault((widx, jg0), []).append(hs[0])
                        else:
                            extras.setdefault((widx, jg0), []).append(hs[0])
                            extras.setdefault((widx, jg1), []).append(hs[1])

                    # widx0: JIT vp[2..15], K-ec0 tb2/3, Q-ec0 ibl1 (K-ec0 tb1
                    # lives in the head). With lookahead-2 emission, extras at
                    # slot k run after AV(k) and after scores(k+2): vp for AV
                    # jg m must sit at slot <= m-1; kpt tb_t (scores jg 2t,
                    # emitted at stage 2t-2) at slot <= 2t-3.
                    sched_block(0, 0, 0, make_vp_halves, 2)
                    sched_block(0, 0, 0, make_vp_halves, 3)
                    sched_block(0, 1, 1, make_vp_halves, 4)
                    sched_block(0, 1, 1, make_vp_halves, 5)
                    sched_block(0, 1, 1, make_proj_halves, kpt, wkt, kvTt, 0, 2)
                    sched_block(0, 2, 2, make_vp_halves, 6)
                    sched_block(0, 2, 2, make_vp_halves, 7)
                    sched_block(0, 3, 3, make_vp_halves, 8)
                    sched_block(0, 3, 3, make_vp_halves, 9)
                    sched_block(0, 3, 3, make_proj_halves, kpt, wkt, kvTt, 0, 3)
                    sched_block(0, 4, 4, make_vp_halves, 10)
                    sched_block(0, 4, 4, make_vp_halves, 11)
                    sched_block(0, 5, 5, make_vp_halves, 12)
                    sched_block(0, 5, 5, make_vp_halves, 13)
                    sched_block(0, 5, 5, make_proj_halves, qpt, wqt, qTt, 0, 1)
                    sched_block(0, 6, 6, make_vp_halves, 14)
                    sched_block(0, 6, 6, make_vp_halves, 15)

                    # widx1..3: Q-ec0 ibl2/3 + pair-1 K/Q projections
                    sched_block(1, 0, 1, make_proj_halves, qpt, wqt, qTt, 0, 2)
                    sched_block(1, 2, 3, make_proj_halves, kpt, wkt, kvTt, 1, 0)
                    sched_block(1, 4, 5, make_proj_halves, kpt, wkt, kvTt, 1, 1)
                    sched_block(1, 6, 7, make_proj_halves, kpt, wkt, kvTt, 1, 2)
                    sched_block(2, 0, 1, make_proj_halves, qpt, wqt, qTt, 0, 3)
                    sched_block(2, 2, 3, make_proj_halves, kpt, wkt, kvTt, 1, 3)
                    sched_block(2, 4, 5, make_proj_halves, qpt, wqt, qTt, 1, 0)
                    sched_block(2, 6, 7, make_proj_halves, qpt, wqt, qTt, 1, 1)
                    sched_block(3, 0, 1, make_proj_halves, qpt, wqt, qTt, 1, 2)
                    sched_block(3, 2, 3, make_proj_halves, qpt, wqt, qTt, 1, 3)

                    # widx4..6 (p1): pair-2 projections
                    sched_block(4, 0, 1, make_proj_halves, kpt, wkt, kvTt, 2, 0)
                    sched_block(4, 2, 3, make_proj_halves, kpt, wkt, kvTt, 2, 1)
                    sched_block(4, 4, 5, make_proj_halves, kpt, wkt, kvTt, 2, 2)
                    sched_block(4, 6, 7, make_proj_halves, kpt, wkt, kvTt, 2, 3)
                    sched_block(5, 0, 1, make_proj_halves, qpt, wqt, qTt, 2, 0)
                    sched_block(5, 2, 3, make_proj_halves, qpt, wqt, qTt, 2, 1)
                    sched_block(5, 4, 5, make_proj_halves, qpt, wqt, qTt, 2, 2)
                    sched_block(5, 6, 7, make_proj_halves, qpt, wqt, qTt, 2, 3)

                    # widx8..9 (p2): pair-3 projections
                    sched_block(8, 0, 1, make_proj_halves, kpt, wkt, kvTt, 3, 0)
                    sched_block(8, 2, 3, make_proj_halves, kpt, wkt, kvTt, 3, 1)
                    sched_block(8, 4, 5, make_proj_halves, kpt, wkt, kvTt, 3, 2)
                    sched_block(8, 6, 7, make_proj_halves, kpt, wkt, kvTt, 3, 3)
                    sched_block(9, 0, 1, make_proj_halves, qpt, wqt, qTt, 3, 0)
                    sched_block(9, 2, 3, make_proj_halves, qpt, wqt, qTt, 3, 1)
                    sched_block(9, 4, 5, make_proj_halves, qpt, wqt, qTt, 3, 2)
                    sched_block(9, 6, 7, make_proj_halves, qpt, wqt, qTt, 3, 3)

                    # widx13..15 (p3 i1..3): out-projection for tb0..2
                    for k, widx in enumerate((13, 14, 15)):
                        for dc in range(D // P):
                            sched_block(widx, dc, dc, make_final_unit, dc, k)

                    # ---------- head: minimal pre-work ----------
                    for h in make_proj_halves(kpt, wkt, kvTt, 0, 0, "pA1"):
                        h()
                    warmup2()
                    for h in make_proj_halves(qpt, wqt, qTt, 0, 0, "pA1"):
                        h()
                    head_tail = []
                    head_tail.extend(make_proj_halves(kpt, wkt, kvTt, 0, 1, "pB1"))
                    head_tail.extend(make_vp_halves(0, "pA1"))
                    head_tail.extend(make_vp_halves(1, "pB1"))

                    # ---------- software-pipelined attention ----------
                    # Per stage (widx, jg): emit scores+exp of the NEXT stage
                    # first (so the PE's in-order stream keeps ScalarE fed
                    # while AV of this stage waits on its exp), then the AV
                    # matmuls, then the interleaved extras, then (at jg==7)
                    # the normalization.
                    def emit_scores_exp(p, ibl, jg):
                        q0 = qpt[p]
                        out = []
                        for j in (2 * jg, 2 * jg + 1):
                            sAB = psS.tile([P, 1024], f32, tag="sAB", name="sAB", bufs=2)
                            nc.tensor.matmul(
                                sAB[:, 0:512],
                                lhsT=kpt[p][0:64, j * P:(j + 1) * P],
                                rhs=q0[0:64, ibl * 512:(ibl + 1) * 512],
                                start=True, stop=True,
                                tile_position=(0, 0),
                            )
                            nc.tensor.matmul(
                                sAB[:, 512:1024],
                                lhsT=kpt[p][64:128, j * P:(j + 1) * P],
                                rhs=q0[64:128, ibl * 512:(ibl + 1) * 512],
                                start=True, stop=True,
                                tile_position=(64, 0),
                            )
                            at = atp.tile([P, 1024], bf16, tag="at", name="at", bufs=6)
                            nc.scalar.activation(at[:, :], sAB[:, :], EXP, scale=SCALE)
                            out.append((j, at))
                        return out

                    powin = {}

                    def emit_avs(p, ibl, pend):
                        widx = p * NTB + ibl
                        if widx not in powin:
                            par = widx % 2
                            powin[widx] = (xps("pA%d" % par), xps("pB%d" % par))
                        po0, po1 = powin[widx]
                        vslc0 = (p * VPW, p * VPW + 65)
                        vslc1 = (p * VPW + 65, (p + 1) * VPW)
                        for j, at in pend:
                            nc.tensor.matmul(
                                po0[0:65, :],
                                lhsT=vp[j][:, vslc0[0]:vslc0[1]],
                                rhs=at[:, 0:512],
                                start=(j == 0), stop=(j == NJT - 1),
                            )
                            nc.tensor.matmul(
                                po1[:, :],
                                lhsT=vp[j][:, vslc1[0]:vslc1[1]],
                                rhs=at[:, 512:1024],
                                start=(j == 0), stop=(j == NJT - 1),
                            )

                    def emit_norm(p, ibl):
                        widx = p * NTB + ibl
                        po0, po1 = powin.pop(widx)
                        srow0 = small.tile([1, 512], f32, tag="srow0", name="srow0", bufs=1)
                        srow1 = small.tile([1, 512], f32, tag="srow1", name="srow1", bufs=1)
                        nc.vector.tensor_copy(out=srow0[:, :], in_=po0[64:65, :])
                        nc.vector.tensor_copy(out=srow1[:, :], in_=po1[0:1, :])
                        posb = small.tile([P, 512], f32, tag="posb", name="posb", bufs=1)
                        nc.vector.tensor_copy(out=posb[0:64, :], in_=po0[0:64, :])
                        nc.vector.tensor_copy(out=posb[64:128, :], in_=po1[64:128, :])
                        rr0 = small.tile([1, 512], f32, tag="rr0", name="rr0", bufs=1)
                        rr1 = small.tile([1, 512], f32, tag="rr1", name="rr1", bufs=1)
                        nc.vector.reciprocal_approx_fast(out=rr0[:, :], in_=srow0[:, :])
                        nc.vector.reciprocal_approx_fast(out=rr1[:, :], in_=srow1[:, :])
                        rbs = small.tile([P, 512], f32, tag="rbs", name="rbs", bufs=1)
                        rbt = small.tile([64, 512], f32, tag="rbt", name="rbt", bufs=1)
                        nc.gpsimd.partition_broadcast(rbs[0:64, :], rr0[0:1, :], channels=64)
                        nc.gpsimd.partition_broadcast(rbt[0:64, :], rr1[0:1, :], channels=64)
                        nc.sync.dma_start(out=rbs[64:128, :], in_=rbt[0:64, :])
                        nc.vector.tensor_tensor(
                            out=onorm[p][0:64, ibl * 512:(ibl + 1) * 512],
                            in0=posb[0:64, :], in1=rbs[0:64, :], op=MULT)
                        nc.vector.tensor_tensor(
                            out=onorm[p][64:128, ibl * 512:(ibl + 1) * 512],
                            in0=posb[64:128, :], in1=rbs[64:128, :], op=MULT)

                    stages = [(p, ibl, jg)
                              for p in range(NPAIR)
                              for ibl in range(NTB)
                              for jg in range(NJT // 2)]
                    pend = {0: emit_scores_exp(*stages[0]),
                            1: emit_scores_exp(*stages[1])}
                    for h in head_tail:
                        h()
                    for i, (p, ibl, jg) in enumerate(stages):
                        if i + 2 < len(stages):
                            pend[i + 2] = emit_scores_exp(*stages[i + 2])
                        emit_avs(p, ibl, pend.pop(i))
                        widx = p * NTB + ibl
                        for fn in extras.get((widx, jg), ()):
                            fn()
                        if jg == NJT // 2 - 1:
                            emit_norm(p, ibl)

                    # ---------- tail: out-projection for tb3, 4-bank spread,
                    # out-DMAs alternating across two DMA engines ----------
                    ttags = ("pA0", "pB0", "pA1", "pB1")
                    for dc in range(D // P):
                        eng = nc.scalar if dc % 2 == 0 else nc.sync
                        make_final_unit(dc, 3, ttags[dc % 4], eng)[0]()

    nc.compile()
    _CACHE["nc"] = nc
    return nc


def _prep_weights(Wq, Wkv, W0):
    bf = ml_dtypes.bfloat16
    per_group = {}
    for g in range(2):
        hg = np.arange(HPC) + g * HPC            # global head ids
        d = np.arange(DH)
        # e_local = h_l*64 + d ; reference maps: e_q = d*16+h, e_k = d*32+h,
        # e_v = d*32+16+h, out channel = h*64+d
        idx_q = (d[None, :] * HEADS + hg[:, None]).reshape(-1)
        idx_k = (d[None, :] * 2 * HEADS + hg[:, None]).reshape(-1)
        idx_v = (d[None, :] * 2 * HEADS + HEADS + hg[:, None]).reshape(-1)
        idx_o = (hg[:, None] * DH + d[None, :]).reshape(-1)
        per_group[g] = {
            "wqT": np.ascontiguousarray(Wq[idx_q, :].T).astype(bf),
            "wkT": np.ascontiguousarray(Wkv[idx_k, :].T).astype(bf),
            "wvT": np.ascontiguousarray(Wkv[idx_v, :].T).astype(bf),
            "w0a": np.ascontiguousarray(W0[:, idx_o].T).astype(bf),
        }
    return per_group


def kernel(q, kv, Wq, Wkv, W0):
    from concourse.bass_utils import run_bass_kernel_spmd

    q = np.asarray(q, dtype=np.float32)
    kv = np.asarray(kv, dtype=np.float32)
    Wq = np.asarray(Wq, dtype=np.float32)
    Wkv = np.asarray(Wkv, dtype=np.float32)
    W0 = np.asarray(W0, dtype=np.float32)

    nc = _build()
    bf = ml_dtypes.bfloat16
    wg = _prep_weights(Wq, Wkv, W0)
    in_maps = []
    for c in range(8):
        b, g = divmod(c, 2)
        in_maps.append({
            "qT": np.ascontiguousarray(q[b].T).astype(bf),
            "kvT": np.ascontiguousarray(kv[b].T).astype(bf),
            "wqT": wg[g]["wqT"],
            "wkT": wg[g]["wkT"],
            "wvT": wg[g]["wvT"],
            "w0a": wg[g]["w0a"],
        })
    trace = bool(int(os.environ.get("KERNEL_TRACE", "0")))
    res = run_bass_kernel_spmd(nc, in_maps, list(range(8)), trace=trace)
    _CACHE["last_result"] = res
    out = np.empty((B, S, D), dtype=np.float32)
    for b in range(B):
        acc = res.results[2 * b]["poutT"] + res.results[2 * b + 1]["poutT"]
        out[b] = acc.T
    return out
